# revision 40
# baseline (speedup 1.0000x reference)
"""Causal self-attention (GQA + RMS-norm + RoPE) Trainium2 Bass kernel.

Sharding: 8 cores = 4 batches x 2 head-groups (tensor-parallel over heads).
Core c = 2*b + t handles batch b with Q heads [8t, 8t+8) and KV heads
[2t, 2t+2). Each core computes a partial output projection (its heads'
rows of W_out); the host sums the two partials per batch.

Design notes (evolved from trace-driven iterations, 964us -> ~535us):
 - all matmul operands bf16 (psum stays f32); qT/kT/v SBUF-resident (no
   DRAM spill round-trip between phases 1 and 2).
 - zero ACT table thrash: phase 1 uses only Square/Sqrt/Copy (one table
   set), phase 2 only Exp; reciprocals via DVE reciprocal_approx_fast.
 - qkv psums drained immediately by ACT (Square+Copy) so banks recycle
   fast; the DVE RoPE chain reads the SBUF copy.
 - phase 2 works in j-PAIRS: two score tiles share a 2-bank psum so one
   [128,1024] exp amortizes ACT access latency; score matmuls are emitted
   one pair ahead ACROSS head/window boundaries (flat unit list) so PE
   never drains.
 - causal masks applied as 0/1 bf16 multiplies on the exp OUTPUT (fast
   all-SBUF DVE mode, off the psum critical path).
 - softmax denominators: DVE adds et pairs into per-4-j partial sums, then
   an all-ones [128,128] stationary matmul broadcasts the total over all
   psum partitions -> normalization is reciprocal+mul on DVE, no PE
   broadcast step.
 - DMA split across sync/scalar/gpsimd queues, x tile 0 first.

Pipeline per core:
  P1: qkv = x @ W_shard (transposed-x input), RMS+RoPE on q/k in natural
      layout, PE-transpose q/k to [d, tok] into resident SBUF.
  P2: per 512-token query window, per head: scoresT = kT_tile.T @ qT_win,
      exp (ACT, scale=hd^-0.5) -> et bf16, 0/1 mask on diag, yT += v.T@et,
      sums via DVE-tree + ones matmul; ytn = yT * (1/sums).
  P3: out = sum_h ytn_h.T @ W_out_h rows -> partial [S, D] f32.
"""
import sys, os
sys.path.insert(0, '/opt/trn_rl_repo')
import numpy as np

from concourse import bass, bacc, mybir, tile

f32 = mybir.dt.float32
f32r = mybir.dt.float32r
bf16 = mybir.dt.bfloat16

B, S, D = 4, 2048, 2048
H, HKV, HD = 16, 4, 128
HLOC = H // 2          # 8 q heads per core
KVLOC = HKV // 2       # 2 kv heads per core
SCALE = float(HD) ** -0.5
ROPE_BASE = 10000.0

NTC = S // 128         # 16 token tiles
NDT = D // 128         # 16 contraction tiles
NWIN = S // 512        # 4 query windows


def _np_bf16():
    import ml_dtypes
    return np.dtype(ml_dtypes.bfloat16)


def _rope_tables():
    inv_freq = (1.0 / (ROPE_BASE ** (np.arange(0, HD, 2, dtype=np.float32) / HD))).astype(np.float32)
    freqs = np.arange(S, dtype=np.float32)[:, None] * inv_freq[None, :]
    cos = np.cos(freqs).astype(np.float32)
    sin = np.sin(freqs).astype(np.float32)
    cos2 = np.concatenate([cos, cos], axis=1)        # [S, 128]
    sin2 = np.concatenate([sin, -sin], axis=1)       # [S, 128]
    return cos2, sin2


def _tri_masks01():
    # mask[vi][p, f] = 0 where kv > q for scoresT diag tiles (else 1):
    # kv = 128*j + p, q = 512*w + f, vi = j - 4*w -> masked iff p + 128*vi > f
    m = np.ones((4, 128, 512), dtype=np.float32)
    p = np.arange(128)[:, None]
    f = np.arange(512)[None, :]
    for vi in range(4):
        m[vi][(p + 128 * vi) > f] = 0.0
    return m


def _emit_rms_rope(nc, scr, psum_ap, nheads, cos1, sin1, nat_tile):
    """psum_ap: [128, nheads*128] qkv psum slice; writes RMS+RoPE result into
    nat_tile (SBUF bf16). cos1/sin1: [128, 1, 128] APs (cos dup, [sin,-sin]).

    rot(q) = q*cos2 + swap_halves(q)*sin2;  out = rot(q) * rsqrt(mean(q^2))
    rsqrt = Sqrt(HD * reciprocal(sum(q^2))) with Square/Sqrt on ACT and the
    reciprocal on DVE -> single activation table set for all of phase 1.
    The psum is drained immediately by the two ACT reads (Square + Copy) so
    its banks recycle fast; the DVE RoPE chain reads the SBUF copy.
    """
    w = nheads * 128
    sq = scr.tile([128, w], f32, tag="sq")
    nc.scalar.activation(sq[:], psum_ap, mybir.ActivationFunctionType.Square)
    qf = scr.tile([128, w], f32, tag="qf")
    nc.scalar.activation(qf[:], psum_ap, mybir.ActivationFunctionType.Copy)
    ss = scr.tile([128, nheads, 1], f32, tag="ss")
    nc.vector.tensor_reduce(
        ss[:], sq[:].rearrange("p (h f) -> p h f", h=nheads),
        axis=mybir.AxisListType.X, op=mybir.AluOpType.add)
    rr = scr.tile([128, nheads, 1], f32, tag="rr")
    nc.vector.reciprocal_approx_fast(rr[:], ss[:])
    rinv = scr.tile([128, nheads, 1, 1], f32, tag="rinv")
    nc.scalar.activation(rinv[:], rr[:], mybir.ActivationFunctionType.Sqrt,
                         scale=float(HD))

    shp = [128, nheads, 2, 64]
    p4 = qf[:].rearrange("p (h x f) -> p h x f", h=nheads, x=2)
    p4s = p4[:, :, ::-1, :]
    cb = cos1.rearrange("p t (x f) -> p t x f", x=2).to_broadcast(shp)
    sb_ = sin1.rearrange("p t (x f) -> p t x f", x=2).to_broadcast(shp)
    rb = rinv[:].to_broadcast(shp)
    t1 = scr.tile(shp, f32, tag="t1")
    t2 = scr.tile(shp, f32, tag="t2")
    nc.vector.tensor_mul(t1[:], p4, cb)
    nc.vector.tensor_mul(t2[:], p4s, sb_)
    nc.vector.tensor_add(t1[:], t1[:], t2[:])
    nc.vector.tensor_mul(nat_tile[:].rearrange("p (h x f) -> p h x f", h=nheads, x=2),
                         t1[:], rb)


def build_program():
    cos_np, sin_np = _rope_tables()
    masks_np = _tri_masks01()
    bf = _np_bf16()

    nc = bacc.Bacc(trn_type="TRN2")

    xt_d = nc.dram_tensor("xt", [D, S], bf16, kind="ExternalInput")
    wq_d = nc.dram_tensor("wq", [D, HLOC * HD], bf16, kind="ExternalInput")
    wkv_d = nc.dram_tensor("wkv", [D, 2 * KVLOC * HD], bf16, kind="ExternalInput")
    wo_d = nc.dram_tensor("wo", [HLOC * HD, D], bf16, kind="ExternalInput")
    out_d = nc.dram_tensor("out", [S, D], bf16, kind="ExternalOutput")

    cos_d = nc.inline_tensor(cos_np.astype(bf), "cos_t")
    sin_d = nc.inline_tensor(sin_np.astype(bf), "sin_t")
    ident_d = nc.inline_tensor(np.eye(128, dtype=np.float32).astype(bf), "ident")
    masks_d = nc.inline_tensor(masks_np.astype(bf), "tri_masks01")
    onescol_d = nc.inline_tensor(np.ones((128, 128), dtype=np.float32).astype(bf), "onescol")

    with tile.TileContext(nc) as tc:
        with tc.tile_pool(name="cst", bufs=1) as cst:
            cos_sb = cst.tile([128, NTC, 128], bf16, tag="cos")
            sin_sb = cst.tile([128, NTC, 128], bf16, tag="sin")
            ident = cst.tile([128, 128], bf16, tag="ident")
            masks01 = cst.tile([128, 4, 512], bf16, tag="masks01")
            ones = cst.tile([128, 128], bf16, tag="ones")
            # resident qT/kT/v (bf16)
            qt_sb = cst.tile([128, HLOC, S], bf16, tag="qt")
            kt_sb = cst.tile([128, KVLOC, NTC, 128], bf16, tag="kt")
            v_sb = cst.tile([128, NTC, KVLOC * HD], bf16, tag="v")

            # tiny consts up front on scalar (before the wq halves); cos/sin/
            # masks go on gpsimd after wkv (not needed until ~12us in)
            nc.scalar.dma_start(out=ident[:], in_=ident_d[:])
            nc.scalar.dma_start(out=ones[:], in_=onescol_d[:])

            # ---------------- Phase 1: QKV projection ----------------
            with tc.tile_pool(name="w1", bufs=1) as w1, \
                 tc.tile_pool(name="xs", bufs=2) as xs, \
                 tc.tile_pool(name="nat", bufs=4) as nat, \
                 tc.tile_pool(name="p1q", bufs=2, space="PSUM") as p1q, \
                 tc.tile_pool(name="p1k", bufs=2, space="PSUM") as p1k, \
                 tc.tile_pool(name="p1t", bufs=2, space="PSUM") as p1t:

                # x tile 0 leads the sync queue, split so the first dt slices
                # land before the whole tile arrives
                xt_tiles = []
                xt_sb0 = xs.tile([128, NDT, 128], bf16, tag="xt")
                x0_r = xt_d[:, 0:128].rearrange("(t p) s -> p t s", p=128)
                for c in range(4):
                    nc.sync.dma_start(out=xt_sb0[:, 4 * c:4 * c + 4, :],
                                      in_=x0_r[:, 4 * c:4 * c + 4, :])
                xt_tiles.append(xt_sb0)

                wq_sb = w1.tile([128, NDT, HLOC * HD], bf16, tag="wq")
                wkv_sb = w1.tile([128, NDT, 512], bf16, tag="wkv")
                wq_r = wq_d[:].rearrange("(t p) c -> p t c", p=128)
                wkv_r = wkv_d[:].rearrange("(t p) c -> p t c", p=128)
                # chunked weight loads: fewer dma_starts -> less DGE
                # descriptor serialization at startup, still dt-ordered
                for c in range(4):
                    lo, hi = 4 * c, 4 * c + 4
                    nc.gpsimd.dma_start(out=wkv_sb[:, lo:hi, :], in_=wkv_r[:, lo:hi, :])
                for c in range(8):
                    lo, hi = 2 * c, 2 * c + 2
                    q_eng = nc.scalar if c % 2 == 0 else nc.sync
                    q_eng.dma_start(out=wq_sb[:, lo:hi, :], in_=wq_r[:, lo:hi, :])
                # bulk consts after wkv on gpsimd
                nc.gpsimd.dma_start(out=cos_sb[:], in_=cos_d[:].rearrange("(t p) f -> p t f", p=128))
                nc.gpsimd.dma_start(out=sin_sb[:], in_=sin_d[:].rearrange("(t p) f -> p t f", p=128))
                nc.gpsimd.dma_start(out=masks01[:], in_=masks_d[:].rearrange("v p f -> p v f"))

                for tcid in range(NTC):
                    if tcid == 0:
                        xt_sb = xt_tiles[0]
                    else:
                        xt_sb = xs.tile([128, NDT, 128], bf16, tag="xt")
                        nc.sync.dma_start(
                            out=xt_sb[:],
                            in_=xt_d[:, tcid * 128:(tcid + 1) * 128]
                                .rearrange("(t p) s -> p t s", p=128))

                    ps_q = p1q.tile([128, 1024], f32, tag="qacc")
                    ps_kv = p1k.tile([128, 512], f32, tag="kvacc")
                    cos1 = cos_sb[:, tcid:tcid + 1, :]
                    sin1 = sin_sb[:, tcid:tcid + 1, :]

                    for dt in range(NDT):
                        st, sp = dt == 0, dt == NDT - 1
                        nc.tensor.matmul(ps_kv[:], xt_sb[:, dt, :], wkv_sb[:, dt, :], start=st, stop=sp)

                    # kv-group RMS/RoPE + v copy emitted BEFORE the q matmul
                    # loop: their ACT ops overlap the q matmuls instead of
                    # piling into the phase-boundary ACT backlog. (The kv
                    # transposes stay at the end so PE never waits on the
                    # DVE rope chain.)
                    kn = nat.tile([128, 256], bf16, tag="kn")
                    _emit_rms_rope(nc, nat, ps_kv[:, 0:256], 2, cos1, sin1, kn)
                    nc.scalar.activation(v_sb[:, tcid, :], ps_kv[:, 256:512],
                                         mybir.ActivationFunctionType.Copy)

                    if tcid < NTC - 1:
                        for dt in range(NDT):
                            st, sp = dt == 0, dt == NDT - 1
                            lhs = xt_sb[:, dt, :]
                            nc.tensor.matmul(ps_q[:, 0:512], lhs, wq_sb[:, dt, 0:512], start=st, stop=sp)
                            nc.tensor.matmul(ps_q[:, 512:1024], lhs, wq_sb[:, dt, 512:1024], start=st, stop=sp)

                        # all 8 q heads at once: RMS+RoPE, PE-transpose to qT
                        qn = nat.tile([128, 1024], bf16, tag="qn")
                        _emit_rms_rope(nc, nat, ps_q[:], 8, cos1, sin1, qn)
                        qsrc = [(qn, h) for h in range(HLOC)]
                    else:
                        # last tile: split q halves so half-1's ACT psum drain
                        # overlaps half-2's matmuls -- shrinks the ACT backlog
                        # that delays window 0's first exps at the phase
                        # boundary
                        for dt in range(NDT):
                            st, sp = dt == 0, dt == NDT - 1
                            nc.tensor.matmul(ps_q[:, 0:512], xt_sb[:, dt, :],
                                             wq_sb[:, dt, 0:512], start=st, stop=sp)
                        qn1 = nat.tile([128, 512], bf16, tag="qn")
                        _emit_rms_rope(nc, nat, ps_q[:, 0:512], 4, cos1, sin1, qn1)
                        for dt in range(NDT):
                            st, sp = dt == 0, dt == NDT - 1
                            nc.tensor.matmul(ps_q[:, 512:1024], xt_sb[:, dt, :],
                                             wq_sb[:, dt, 512:1024], start=st, stop=sp)
                        qn2 = nat.tile([128, 512], bf16, tag="qn")
                        _emit_rms_rope(nc, nat, ps_q[:, 512:1024], 4, cos1, sin1, qn2)
                        qsrc = [(qn1, h) for h in range(4)] + [(qn2, h - 4) for h in range(4, HLOC)]
                    for h, (src_t, off) in enumerate(qsrc):
                        tp = p1t.tile([128, 128], bf16, tag="tp")
                        nc.tensor.transpose(tp[:], src_t[:, off * 128:(off + 1) * 128], ident[:])
                        nc.vector.tensor_copy(
                            qt_sb[:, h, tcid * 128:(tcid + 1) * 128], tp[:])
                    for kh in range(KVLOC):
                        tp = p1t.tile([128, 128], bf16, tag="tp")
                        nc.tensor.transpose(tp[:], kn[:, kh * 128:(kh + 1) * 128], ident[:])
                        nc.vector.tensor_copy(
                            kt_sb[:, kh, tcid, :], tp[:])

            # ------------- Phases 2+3 (share the ytn resident) -------------
            with tc.tile_pool(name="ytp", bufs=1) as ytp:
                ytn = ytp.tile([128, HLOC, S], bf16, tag="ytn")

                with tc.tile_pool(name="w3", bufs=1) as w3:
                    # W_out loaded during phase 2 (needed at phase 3 start)
                    wo_sb = w3.tile([128, HLOC, D], bf16, tag="wo")
                    wo_r = wo_d[:].rearrange("(h p) c -> p h c", p=128)
                    for og in range(4):
                        nc.gpsimd.dma_start(out=wo_sb[:, :, og * 512:(og + 1) * 512],
                                            in_=wo_r[:, :, og * 512:(og + 1) * 512])

                    # ---- Phase 2: attention ----
                    with tc.tile_pool(name="ex", bufs=6) as ex, \
                         tc.tile_pool(name="sm", bufs=6) as sm, \
                         tc.tile_pool(name="p2s", bufs=2, space="PSUM") as p2s, \
                         tc.tile_pool(name="p2y", bufs=2, space="PSUM") as p2y, \
                         tc.tile_pool(name="p2n", bufs=2, space="PSUM") as p2n:

                        # flat unit list: one unit = one j-pair of one head;
                        # sc matmuls emitted one unit ahead ACROSS head and
                        # window boundaries so PE never drains
                        units = []
                        for w in range(NWIN):
                            for hq in range(HLOC):
                                for pi in range((4 * w + 4) // 2):
                                    units.append((w, hq, pi))

                        def emit_sc_pair(unit):
                            w, hq, pi = unit
                            kvh = hq // 4
                            rhs_q = qt_sb[:, hq, w * 512:(w + 1) * 512]
                            ps_sc = p2s.tile([128, 2, 512], f32, tag="sc")
                            for half in range(2):
                                j = 2 * pi + half
                                nc.tensor.matmul(
                                    ps_sc[:, half, :],
                                    kt_sb[:, kvh, j, :], rhs_q)
                            return ps_sc

                        head_state = {}   # (w, hq) -> (ps_y, ps_s, pending half-sums)
                        sc_pend = {units[0]: emit_sc_pair(units[0])}
                        for ui, unit in enumerate(units):
                            if ui + 1 < len(units):
                                nxt = units[ui + 1]
                                sc_pend[nxt] = emit_sc_pair(nxt)
                            w, hq, pi = unit
                            njt = 4 * w + 4
                            npair = njt // 2
                            kvh = hq // 4
                            if pi == 0:
                                ps_y = p2y.tile([128, 512], f32, tag="y")
                                ps_s = p2n.tile([128, 512], f32, tag="s")
                                head_state[(w, hq)] = (ps_y, ps_s, [])
                            ps_y, ps_s, hsums = head_state[(w, hq)]

                            ps_sc = sc_pend.pop(unit)
                            et = ex.tile([128, 2, 512], bf16, tag="et")
                            nc.scalar.activation(et[:], ps_sc[:],
                                                 mybir.ActivationFunctionType.Exp,
                                                 scale=SCALE)
                            if 2 * pi >= 4 * w:
                                # 0/1 bf16 mask multiply on et (off the psum
                                # path; DVE fast mode on all-SBUF bf16)
                                vi = 2 * pi - 4 * w
                                em = ex.tile([128, 2, 512], bf16, tag="em")
                                nc.vector.tensor_mul(em[:], et[:],
                                                     masks01[:, vi:vi + 2, :])
                                et = em
                            for half in range(2):
                                j = 2 * pi + half
                                nc.tensor.matmul(
                                    ps_y[:],
                                    v_sb[:, j, kvh * 128:(kvh + 1) * 128],
                                    et[:, half, :], start=(j == 0), stop=(j == njt - 1),
                                    skip_group_check=True)
                            # denominator tree: et halves -> pair sum; two
                            # pair sums -> one PE sums matmul per 4 j's
                            hs = sm.tile([128, 512], bf16, tag="hs")
                            nc.vector.tensor_add(hs[:], et[:, 0, :], et[:, 1, :])
                            hsums.append(hs)
                            if len(hsums) == 2:
                                qs = sm.tile([128, 512], bf16, tag="qs")
                                nc.vector.tensor_add(qs[:], hsums[0][:], hsums[1][:])
                                nc.tensor.matmul(
                                    ps_s[:], ones[:], qs[:],
                                    start=(pi == 1), stop=(pi == npair - 1),
                                    skip_group_check=True)
                                hsums.clear()
                            if pi == npair - 1:
                                rec = sm.tile([128, 512], f32, tag="rec")
                                nc.vector.reciprocal_approx_fast(rec[:], ps_s[:])
                                nc.vector.tensor_mul(
                                    ytn[:, hq, w * 512:(w + 1) * 512], ps_y[:], rec[:])
                                del head_state[(w, hq)]

                    # ---- Phase 3: output projection ----
                    with tc.tile_pool(name="ob", bufs=4) as ob, \
                         tc.tile_pool(name="p3", bufs=4, space="PSUM") as p3:
                        for tcid in range(NTC):
                            for og in range(4):
                                ps_o = p3.tile([128, 512], f32, tag="o")
                                for h in range(HLOC):
                                    nc.tensor.matmul(
                                        ps_o[:],
                                        ytn[:, h, tcid * 128:(tcid + 1) * 128],
                                        wo_sb[:, h, og * 512:(og + 1) * 512],
                                        start=(h == 0), stop=(h == HLOC - 1))
                                ot = ob.tile([128, 512], bf16, tag="ot")
                                nc.scalar.activation(ot[:], ps_o[:],
                                                     mybir.ActivationFunctionType.Copy)
                                q_eng = nc.gpsimd if og % 2 == 0 else nc.sync
                                q_eng.dma_start(
                                    out=out_d[tcid * 128:(tcid + 1) * 128, og * 512:(og + 1) * 512],
                                    in_=ot[:])

    nc.compile()
    return nc


_PROGRAM = None


def _get_program():
    global _PROGRAM
    if _PROGRAM is None:
        _PROGRAM = build_program()
    return _PROGRAM


def make_in_maps(x, W_qkv, W_out):
    bf = _np_bf16()
    x = np.asarray(x, dtype=np.float32)
    W_qkv = np.asarray(W_qkv, dtype=np.float32)
    W_out = np.asarray(W_out, dtype=np.float32)
    in_maps = []
    for c in range(8):
        b, t = c // 2, c % 2
        xt = np.ascontiguousarray(x[b].T).astype(bf)
        wq = np.ascontiguousarray(W_qkv[:, t * 1024:(t + 1) * 1024]).astype(bf)
        wk = W_qkv[:, D + t * 256: D + (t + 1) * 256]
        wv = W_qkv[:, D + 512 + t * 256: D + 512 + (t + 1) * 256]
        wkv = np.ascontiguousarray(np.concatenate([wk, wv], axis=1)).astype(bf)
        wo = np.ascontiguousarray(W_out[t * 1024:(t + 1) * 1024, :]).astype(bf)
        in_maps.append({"xt": xt, "wq": wq, "wkv": wkv, "wo": wo})
    return in_maps


def kernel(x, W_qkv, W_out):
    from concourse.bass_utils import run_bass_kernel_spmd
    nc = _get_program()
    in_maps = make_in_maps(x, W_qkv, W_out)
    res = run_bass_kernel_spmd(nc, in_maps, list(range(8)), trace=False)
    out = np.empty((B, S, D), dtype=np.float32)
    for b in range(B):
        out[b] = (res.results[2 * b]["out"].astype(np.float32)
                  + res.results[2 * b + 1]["out"].astype(np.float32))
    return out
